# revision 1
# baseline (speedup 1.0000x reference)
"""PointPillarsScatter on 8 TRN2 NeuronCores.

Reference op: scatter N pillar feature vectors [N, 64] into a canvas
[B=4, C=64, NY=496, NX=432] at (y, x) cell coords (zero elsewhere).

Sharding: 8 cores = 4 batches x 2 y-halves. Core k=(b, g) owns the
canvas slice out[b, :, 248*g : 248*(g+1), :] -> flat [64, 107136].

Device algorithm (per core), all standard engine ops:
  - canvas is produced in column-windows of W=512 cells across 2
    column-slabs stacked on partitions: window tile [128, 512] where
    partition p = 64*a + c (a = slab, c = channel).
  - for each window, host packs the <=128 pillars that land in it into
    "slots": lhsT weights [128 slots, 128] with w[k, 64*slab_k + c] =
    feat[pillar_k, c], and a local column index idx[k] in [0, 512).
  - DVE builds onehot[k, j] = (iota[j] == idx[k]) with one tensor_scalar.
  - PE matmul lhsT.T @ onehot -> PSUM [128, 512] = the scattered window
    (empty cells read exact 0.0; occupied cells the exact f32 feature
    since onehot rows are 0/1 and products/sums are exact).
  - copy PSUM -> SBUF (alternating DVE/ACT), accumulate SUPER=8 windows
    into one [128, 4096] tile, DMA it to a CONTIGUOUS DRAM superblock
    (scattered multi-descriptor DMA patterns measured ~10x below line
    rate; contiguous superblocks merge descriptors to full rate).
  - host unscrambles superblocks into the final canvas layout.

Self-contained: shapes hardcoded, no sibling imports.
"""

import numpy as np

NY, NX, C = 496, 432, 64
B = 4
N_CORES = 8
HALF_Y = NY // 2  # 248
CORE_COLS = HALF_Y * NX  # 107136 canvas cells per core
SLABS = 2
SLAB = CORE_COLS // SLABS  # 53568
W = 512  # window width (canvas cells per matmul)
NWIN = (SLAB + W - 1) // W  # 105 windows (last = 320 cols)
LAST_W = SLAB - (NWIN - 1) * W  # 320
SLOTS = 64  # pillar slots per slab per matmul chunk (slab a owns
            # partitions [64a, 64a+64) of the slot space)
GROUP = 16  # weight-tile entries fetched per input DMA
SUPER = 4  # windows per output superblock DMA
NSB = NWIN // SUPER  # 13 full superblocks; remainder windows after that
REM_WINS = NWIN - NSB * SUPER  # 1 (the 320-col window)
OUT_ELEMS = C * CORE_COLS  # per-core output element count

_cache = {}


def _build_program(chunks_per_window, nwt, repeat=1, mode="full",
                   psum_bufs=6, oh_bufs=4, sb_bufs=4, wt_bufs=3,
                   copy_mode="act", super_w=SUPER, group=GROUP,
                   cmp_split=False, oh_bf16=False):
    """Build the shared SPMD bass program for the given window schedule.

    chunks_per_window: list[int] of length NWIN (>=1 each), shared by all
    cores. nwt == sum(chunks_per_window) weight-tile entries.
    mode: "full" | "dmaonly" (skip compute, DMA a constant tile) |
    "nodma" (compute, tiny out-DMA only) — bisection benchmarks.
    """
    import concourse.bacc as bacc
    import concourse.bass as bass
    import concourse.tile as tile
    import concourse.mybir as mybir
    from contextlib import ExitStack

    f32 = mybir.dt.float32

    nc = bacc.Bacc("TRN2", target_bir_lowering=False, debug=False,
                   num_devices=N_CORES)

    w_dram = nc.dram_tensor("w", [128, nwt * C], f32, kind="ExternalInput")
    idx_dram = nc.dram_tensor("idx", [128, nwt], f32, kind="ExternalInput")
    iota_dram = nc.dram_tensor("iota", [128, W], f32, kind="ExternalInput")
    # scrambled output: NSB superblocks [128, SUPER*W] + remainder windows
    out_dram = nc.dram_tensor("out", [1, OUT_ELEMS], f32, kind="ExternalOutput")

    SUP = super_w
    NSB_L = NWIN // SUP
    with tile.TileContext(nc) as tc, ExitStack() as ctx:
        const_pool = ctx.enter_context(tc.tile_pool(name="const", bufs=1))
        w_pool = ctx.enter_context(tc.tile_pool(name="wpool", bufs=wt_bufs))
        oh_pool = ctx.enter_context(tc.tile_pool(name="ohpool", bufs=oh_bufs))
        out_pool = ctx.enter_context(tc.tile_pool(name="opool", bufs=sb_bufs))
        psum_pool = ctx.enter_context(
            tc.tile_pool(name="pspool", bufs=psum_bufs, space="PSUM"))

        iota_t = const_pool.tile([128, W], f32)
        nc.sync.dma_start(iota_t[:], iota_dram.ap())
        idx_t = const_pool.tile([128, nwt], f32)
        nc.sync.dma_start(idx_t[:], idx_dram.ap())
        zed = None
        if mode == "dmaonly":
            zed = const_pool.tile([128, SUP * W], f32)
            nc.vector.memset(zed[:], 0.125)

        w_ap = w_dram.ap()

        for rep in range(repeat):
            e = 0
            w_tiles = {}
            sb_tile = None
            sb_base = 0  # first window index of current superblock
            for w in range(NWIN):
                n = W if w < NWIN - 1 else LAST_W
                in_super = w < NSB_L * SUP
                if in_super and w % SUP == 0:
                    sb_tile = out_pool.tile([128, SUP * W], f32, tag="sb",
                                            name=f"sb_{rep}_{w // SUP}")
                    sb_base = w
                nchunks = chunks_per_window[w] if mode != "dmaonly" else 0
                ps = psum_pool.tile([128, W], f32, tag="ps",
                                    name=f"ps_{rep}_{w}")
                for t in range(nchunks):
                    g = e // group
                    if g not in w_tiles:
                        glen = min(group, nwt - g * group)
                        wt = w_pool.tile([128, group * 128], f32, tag="wt",
                                         name=f"wt_{rep}_{g}")
                        # zero the tile (GPSIMD, otherwise idle), then the
                        # load DMA expands dense [128, e*64] weights into the
                        # block-diagonal layout: slot partition p = 64u+v
                        # lands at free offset i*128 + 64u + c (affine in
                        # (u, v, i, c) so a single 4D DMA does it).
                        nc.gpsimd.memset(wt[:], 0.0)
                        FW = group * 128
                        for u in range(2):
                            dst = bass.AP(wt.tensor,
                                          wt.offset + u * (64 * FW + 64),
                                          [[FW, 64], [128, glen], [1, C]])
                            src = bass.AP(w_dram,
                                          g * group * C + u * 64 * nwt * C,
                                          [[nwt * C, 64], [C, glen], [1, C]])
                            nc.gpsimd.dma_start(dst, src)
                        w_tiles[g] = wt
                    wt = w_tiles[g]
                    woff = (e % group) * 128
                    # plain fp32 matmul (4 cycles/row): float32r runs
                    # 4x faster but is reduced precision on HW (measured
                    # absmax 1e-3) — this op must be bit-exact.
                    oh_dt = mybir.dt.bfloat16 if oh_bf16 else f32
                    oh = oh_pool.tile([128, W], oh_dt, tag="oh",
                                      name=f"oh_{rep}_{w}_{t}")
                    cmp_eng = nc.gpsimd if (cmp_split and w % 3 == 2) \
                        else nc.vector
                    cmp_eng.tensor_scalar(
                        oh[:, :n], iota_t[:, :n], idx_t[:, e : e + 1], None,
                        op0=mybir.AluOpType.is_equal)
                    nc.tensor.matmul(
                        ps[:, :n], wt[:, woff : woff + 128], oh[:, :n],
                        start=(t == 0), stop=(t == nchunks - 1))
                    e += 1
                if in_super:
                    j0 = (w - sb_base) * W
                    dstslice = sb_tile[:, j0 : j0 + n]
                else:
                    sb_tile = out_pool.tile([128, SUP * W], f32, tag="sb",
                                            name=f"sb_{rep}_r{w}")
                    dstslice = sb_tile[:, :n]
                if mode != "dmaonly":
                    # PSUM->SBUF copies: alternate DVE/ACT or pin one engine
                    use_v = (w % 2 == 0) if copy_mode == "alt" else (
                        copy_mode == "dve")
                    if use_v:
                        nc.vector.tensor_copy(dstslice, ps[:, :n])
                    else:
                        nc.scalar.copy(dstslice, ps[:, :n])
                if mode == "nodma":
                    off = w * 128 * 16
                    dst = bass.AP(out_dram, off, [[16, 128], [1, 16]])
                    nc.sync.dma_start(dst, sb_tile[:, :16])
                    continue
                src_tile = sb_tile if mode != "dmaonly" else zed
                if in_super and (w - sb_base) == SUP - 1:
                    off = sb_base * 128 * W
                    dst = bass.AP(out_dram, off, [[SUP * W, 128],
                                                  [1, SUP * W]])
                    nc.sync.dma_start(dst, src_tile[:])
                elif not in_super:
                    off = NSB_L * SUP * 128 * W + (w - NSB_L * SUP) * 128 * LAST_W
                    dst = bass.AP(out_dram, off, [[n, 128], [1, n]])
                    nc.sync.dma_start(dst, src_tile[:, :n])
            assert e == nwt or mode == "dmaonly"

    nc.compile()
    return nc


def _unscramble(core_flat):
    """[OUT_ELEMS] scrambled superblocks -> canvas [C, CORE_COLS]."""
    canvas = np.empty((C, CORE_COLS), dtype=np.float32)
    main = core_flat[: NSB * 128 * SUPER * W].reshape(
        NSB, SLABS, C, SUPER * W)  # [g, a, c, j]
    # canvas cols a*SLAB + g*SUPER*W + j  for j in [0, SUPER*W)
    m = main.transpose(2, 1, 0, 3).reshape(C, SLABS, NSB * SUPER * W)
    canvas_v = canvas.reshape(C, SLABS, SLAB)
    canvas_v[:, :, : NSB * SUPER * W] = m
    off = NSB * 128 * SUPER * W
    for r in range(REM_WINS):
        w = NSB * SUPER + r
        blk = core_flat[off : off + 128 * LAST_W].reshape(SLABS, C, LAST_W)
        canvas_v[:, :, w * W : w * W + LAST_W] = blk.transpose(1, 0, 2)
        off += 128 * LAST_W
    return canvas


def _host_pack(voxel_features, coords):
    """Shard + pack inputs for the 8 cores.

    Returns (in_maps, chunks_per_window, nwt).
    """
    vf = np.ascontiguousarray(np.asarray(voxel_features, dtype=np.float32))
    cd = np.asarray(coords)
    bidx = cd[:, 0].astype(np.int64)
    yy = cd[:, 2].astype(np.int64)
    xx = cd[:, 3].astype(np.int64)

    # jax scatter drops out-of-bounds indices; match by masking them out
    inb = (yy >= 0) & (yy < NY) & (xx >= 0) & (xx < NX)

    cores = []
    counts_per_core = []
    for b in range(B):
        for g in range(2):
            sel = np.nonzero(inb & (bidx == b) & (yy >= g * HALF_Y)
                             & (yy < (g + 1) * HALF_Y))[0]
            flat = (yy[sel] - g * HALF_Y) * NX + xx[sel]  # [0, CORE_COLS)
            # dedupe duplicate cells, keep the LAST occurrence
            if len(flat):
                u_rev, first_rev = np.unique(flat[::-1], return_index=True)
                keep = len(flat) - 1 - first_rev
                sel, flat = sel[keep], flat[keep]
            slab = flat // SLAB
            within = flat % SLAB
            win = within // W
            loc = within % W
            # slot space: per (window, slab); slab a owns partitions
            # [64a, 64a+64) and chunk t covers slots [64t, 64t+64) there
            key = win * SLABS + slab
            order = np.argsort(key, kind="stable")
            sel, slab, win, loc = sel[order], slab[order], win[order], loc[order]
            key = key[order]
            kcounts = np.bincount(key, minlength=NWIN * SLABS)
            starts = np.concatenate([[0], np.cumsum(kcounts)[:-1]])
            slot_within = np.arange(len(win)) - starts[key]
            cores.append((sel, slab, win, loc, slot_within))
            counts_per_core.append(kcounts)

    counts_max = np.max(np.stack(counts_per_core), axis=0).reshape(NWIN, SLABS)
    counts_max = counts_max.max(axis=1)  # worst slab per window
    chunks_per_window = np.maximum(1, -(-counts_max // SLOTS)).astype(np.int64)
    nwt = int(chunks_per_window.sum())
    entry0 = np.concatenate([[0], np.cumsum(chunks_per_window)[:-1]])

    iota = np.tile(np.arange(W, dtype=np.float32), (128, 1))

    in_maps = []
    for (sel, slab, win, loc, slot_within) in cores:
        chunk = slot_within // SLOTS
        slot = (SLOTS * slab + slot_within % SLOTS).astype(np.int64)
        entry = entry0[win] + chunk
        wt = np.zeros((nwt, 128, C), dtype=np.float32)
        idxc = np.full((nwt, 128), -1.0, dtype=np.float32)
        if len(sel):
            wt[entry, slot] = vf[sel]
            idxc[entry, slot] = loc.astype(np.float32)
        w_dev = np.ascontiguousarray(
            wt.transpose(1, 0, 2).reshape(128, nwt * C))
        idx_dev = np.ascontiguousarray(idxc.T)
        in_maps.append({"w": w_dev, "idx": idx_dev, "iota": iota})

    return in_maps, tuple(int(c) for c in chunks_per_window), nwt


def _run(voxel_features, coords, trace=False):
    from concourse.bass_utils import run_bass_kernel_spmd

    in_maps, chunks, nwt = _host_pack(voxel_features, coords)
    key = chunks
    if key not in _cache:
        _cache[key] = _build_program(chunks, nwt)
    nc = _cache[key]

    res = run_bass_kernel_spmd(nc, in_maps, core_ids=list(range(N_CORES)),
                               trace=trace)
    out = np.zeros((B, C, NY, NX), dtype=np.float32)
    for k in range(N_CORES):
        b, g = divmod(k, 2)
        canvas = _unscramble(res.results[k]["out"].reshape(-1))
        out[b, :, g * HALF_Y : (g + 1) * HALF_Y, :] = canvas.reshape(
            C, HALF_Y, NX)
    return out, res


def kernel(voxel_features, coords, batch_size=B):
    assert int(batch_size) == B
    out, _ = _run(voxel_features, coords, trace=False)
    return out



# revision 3
# speedup vs baseline: 1.0456x; 1.0456x over previous
"""PointPillarsScatter on 8 TRN2 NeuronCores.

Reference op: scatter N pillar feature vectors [N, 64] into a canvas
[B=4, C=64, NY=496, NX=432] at (y, x) cell coords (zero elsewhere).

Sharding: 8 cores = 4 batches x 2 y-halves. Core k=(b, g) owns the
canvas slice out[b, :, 248*g : 248*(g+1), :] -> flat [64, 107136].

Device algorithm (per core), all standard engine ops:
  - canvas is produced in column-windows of W=512 cells across 2
    column-slabs stacked on partitions: window tile [128, 512] where
    partition p = 64*a + c (a = slab, c = channel).
  - for each window, host packs the <=64 pillars per slab that land in
    it into "slots": dense fp16 weights w[slot, e*64 + c] =
    feat[pillar_k, c] (slot = 64*slab + within-slab slot), and a local
    column index idx[slot] in [0, 512).
  - DVE builds onehot[k, j] = (iota[j] == idx[k]) in fp16 (ints <= 512
    are exact in fp16; the f32 scalar compare is exact).
  - PE: one matmul per slab, lhsT = dense fp16 weights [64 slots, 64 ch],
    rhs = onehot[64 slots, 512], accumulating into PSUM partitions
    [64a, 64a+64).  fp16 matmul streams 1 cycle/column vs 4 for fp32,
    and the dense lhsT removes the block-diagonal weight expansion
    (and its strided sub-512B-descriptor DMAs) entirely.  Products are
    exact fp16 values widened in fp32 PSUM; each canvas cell receives
    exactly one product, so occupied cells equal fp16(feature) (rel err
    ~2^-11, far under the 2e-2 gate) and empty cells exact 0.0.
  - copy PSUM -> SBUF f32 (round-robin ACT/DVE/Pool so no engine
    exceeds the DMA-limited window cadence), accumulate SUPER=4 windows
    into one [128, 2048] tile, DMA it to a CONTIGUOUS DRAM superblock
    (scattered multi-descriptor DMA patterns run far below line rate;
    contiguous superblocks with 8KB descriptors hit full rate).
  - host unscrambles superblocks into the final canvas layout.

Self-contained: shapes hardcoded, no sibling imports.
"""

import numpy as np

NY, NX, C = 496, 432, 64
B = 4
N_CORES = 8
HALF_Y = NY // 2  # 248
CORE_COLS = HALF_Y * NX  # 107136 canvas cells per core
SLABS = 2
SLAB = CORE_COLS // SLABS  # 53568
W = 512  # window width (canvas cells per matmul)
NWIN = (SLAB + W - 1) // W  # 105 windows (last = 320 cols)
LAST_W = SLAB - (NWIN - 1) * W  # 320
SLOTS = 64  # pillar slots per slab per matmul chunk (slab a owns
            # partitions [64a, 64a+64) of the slot space)
GROUP = 16  # weight-tile entries fetched per input DMA
SUPER = 4  # windows per output superblock DMA
NSB = NWIN // SUPER  # 26 full superblocks; remainder windows after that
REM_WINS = NWIN - NSB * SUPER  # 1 (the 320-col window)
OUT_ELEMS = C * CORE_COLS  # per-core output element count

_cache = {}


def _build_program(chunks_per_window, nwt, repeat=1, mode="full",
                   psum_bufs=6, oh_bufs=4, sb_bufs=4, wt_bufs=3,
                   super_w=SUPER, group=GROUP):
    """Build the shared SPMD bass program for the given window schedule.

    chunks_per_window: list[int] of length NWIN (>=1 each), shared by all
    cores. nwt == sum(chunks_per_window) weight-tile entries.
    mode: "full" | "dmaonly" (skip compute, DMA a constant tile) |
    "nodma" (compute, tiny out-DMA only) — bisection benchmarks.
    """
    import concourse.bacc as bacc
    import concourse.bass as bass
    import concourse.tile as tile
    import concourse.mybir as mybir
    from contextlib import ExitStack

    f32 = mybir.dt.float32
    f16 = mybir.dt.float16

    nc = bacc.Bacc("TRN2", target_bir_lowering=False, debug=False,
                   num_devices=N_CORES)

    # idx padded to 128 entries-multiple so its load descriptors are >=512B
    nwt_p = -(-nwt // 128) * 128
    w_dram = nc.dram_tensor("w", [128, nwt * C], f16, kind="ExternalInput")
    idx_dram = nc.dram_tensor("idx", [128, nwt_p], f32, kind="ExternalInput")
    iota_dram = nc.dram_tensor("iota", [128, W], f16, kind="ExternalInput")
    # scrambled output: NSB superblocks [128, SUPER*W] + remainder windows
    out_dram = nc.dram_tensor("out", [1, OUT_ELEMS], f32, kind="ExternalOutput")

    SUP = super_w
    NSB_L = NWIN // SUP
    with tile.TileContext(nc) as tc, ExitStack() as ctx:
        const_pool = ctx.enter_context(tc.tile_pool(name="const", bufs=1))
        w_pool = ctx.enter_context(tc.tile_pool(name="wpool", bufs=wt_bufs))
        oh_pool = ctx.enter_context(tc.tile_pool(name="ohpool", bufs=oh_bufs))
        out_pool = ctx.enter_context(tc.tile_pool(name="opool", bufs=sb_bufs))
        psum_pool = ctx.enter_context(
            tc.tile_pool(name="pspool", bufs=psum_bufs, space="PSUM"))

        iota_t = const_pool.tile([128, W], f16)
        nc.sync.dma_start(iota_t[:], iota_dram.ap())
        idx_t = const_pool.tile([128, nwt_p], f32)
        nc.sync.dma_start(idx_t[:], idx_dram.ap())
        zed = None
        if mode == "dmaonly":
            zed = const_pool.tile([128, SUP * W], f32)
            nc.vector.memset(zed[:], 0.125)

        for rep in range(repeat):
            e = 0
            w_tiles = {}
            sb_tile = None
            sb_base = 0  # first window index of current superblock
            for w in range(NWIN):
                n = W if w < NWIN - 1 else LAST_W
                in_super = w < NSB_L * SUP
                if in_super and w % SUP == 0:
                    sb_tile = out_pool.tile([128, SUP * W], f32, tag="sb",
                                            name=f"sb_{rep}_{w // SUP}")
                    sb_base = w
                nchunks = chunks_per_window[w] if mode != "dmaonly" else 0
                ps = psum_pool.tile([128, W], f32, tag="ps",
                                    name=f"ps_{rep}_{w}")
                for t in range(nchunks):
                    g = e // group
                    if g not in w_tiles:
                        glen = min(group, nwt - g * group)
                        wt = w_pool.tile([128, group * C], f16, tag="wt",
                                         name=f"wt_{rep}_{g}")
                        # dense fp16 weights, contiguous load (2KB/desc)
                        src = bass.AP(w_dram, g * group * C,
                                      [[nwt * C, 128], [1, glen * C]])
                        nc.sync.dma_start(wt[:, : glen * C], src)
                        w_tiles[g] = wt
                    wt = w_tiles[g]
                    woff = (e % group) * C
                    oh = oh_pool.tile([128, W], f16, tag="oh",
                                      name=f"oh_{rep}_{w}_{t}")
                    nc.vector.tensor_scalar(
                        oh[:, :n], iota_t[:, :n], idx_t[:, e : e + 1], None,
                        op0=mybir.AluOpType.is_equal)
                    for a in range(SLABS):
                        p0 = SLOTS * a
                        nc.tensor.matmul(
                            ps[p0 : p0 + C, :n],
                            wt[p0 : p0 + SLOTS, woff : woff + C],
                            oh[p0 : p0 + SLOTS, :n],
                            start=(t == 0), stop=(t == nchunks - 1))
                    e += 1
                if in_super:
                    j0 = (w - sb_base) * W
                    dstslice = sb_tile[:, j0 : j0 + n]
                else:
                    sb_tile = out_pool.tile([128, SUP * W], f32, tag="sb",
                                            name=f"sb_{rep}_r{w}")
                    dstslice = sb_tile[:, :n]
                if mode != "dmaonly":
                    # PSUM->SBUF copies alternate ACT/DVE (GPSIMD cannot
                    # read PSUM): each stays under the ~730ns DMA-limited
                    # window cadence
                    if w % 2 == 0:
                        nc.scalar.copy(dstslice, ps[:, :n])
                    else:
                        nc.vector.tensor_copy(dstslice, ps[:, :n])
                if mode == "nodma":
                    off = w * 128 * 16
                    dst = bass.AP(out_dram, off, [[16, 128], [1, 16]])
                    nc.sync.dma_start(dst, sb_tile[:, :16])
                    continue
                src_tile = sb_tile if mode != "dmaonly" else zed
                if in_super and (w - sb_base) == SUP - 1:
                    off = sb_base * 128 * W
                    dst = bass.AP(out_dram, off, [[SUP * W, 128],
                                                  [1, SUP * W]])
                    nc.sync.dma_start(dst, src_tile[:])
                elif not in_super:
                    off = NSB_L * SUP * 128 * W + (w - NSB_L * SUP) * 128 * LAST_W
                    dst = bass.AP(out_dram, off, [[n, 128], [1, n]])
                    nc.sync.dma_start(dst, src_tile[:, :n])
            assert e == nwt or mode == "dmaonly"

    nc.compile()
    return nc


def _unscramble(core_flat):
    """[OUT_ELEMS] scrambled superblocks -> canvas [C, CORE_COLS]."""
    canvas = np.empty((C, CORE_COLS), dtype=np.float32)
    main = core_flat[: NSB * 128 * SUPER * W].reshape(
        NSB, SLABS, C, SUPER * W)  # [g, a, c, j]
    # canvas cols a*SLAB + g*SUPER*W + j  for j in [0, SUPER*W)
    m = main.transpose(2, 1, 0, 3).reshape(C, SLABS, NSB * SUPER * W)
    canvas_v = canvas.reshape(C, SLABS, SLAB)
    canvas_v[:, :, : NSB * SUPER * W] = m
    off = NSB * 128 * SUPER * W
    for r in range(REM_WINS):
        w = NSB * SUPER + r
        blk = core_flat[off : off + 128 * LAST_W].reshape(SLABS, C, LAST_W)
        canvas_v[:, :, w * W : w * W + LAST_W] = blk.transpose(1, 0, 2)
        off += 128 * LAST_W
    return canvas


def _host_pack(voxel_features, coords):
    """Shard + pack inputs for the 8 cores.

    Returns (in_maps, chunks_per_window, nwt).
    """
    vf = np.asarray(voxel_features, dtype=np.float32)
    cd = np.asarray(coords)
    bidx = cd[:, 0].astype(np.int64)
    yy = cd[:, 2].astype(np.int64)
    xx = cd[:, 3].astype(np.int64)

    # jax scatter drops out-of-bounds indices; match by masking them out
    inb = (yy >= 0) & (yy < NY) & (xx >= 0) & (xx < NX)

    cores = []
    counts_per_core = []
    for b in range(B):
        for g in range(2):
            sel = np.nonzero(inb & (bidx == b) & (yy >= g * HALF_Y)
                             & (yy < (g + 1) * HALF_Y))[0]
            flat = (yy[sel] - g * HALF_Y) * NX + xx[sel]  # [0, CORE_COLS)
            # dedupe duplicate cells, keep the LAST occurrence
            if len(flat):
                u_rev, first_rev = np.unique(flat[::-1], return_index=True)
                keep = len(flat) - 1 - first_rev
                sel, flat = sel[keep], flat[keep]
            slab = flat // SLAB
            within = flat % SLAB
            win = within // W
            loc = within % W
            # slot space: per (window, slab); slab a owns partitions
            # [64a, 64a+64) and chunk t covers slots [64t, 64t+64) there
            key = win * SLABS + slab
            order = np.argsort(key, kind="stable")
            sel, slab, win, loc = sel[order], slab[order], win[order], loc[order]
            key = key[order]
            kcounts = np.bincount(key, minlength=NWIN * SLABS)
            starts = np.concatenate([[0], np.cumsum(kcounts)[:-1]])
            slot_within = np.arange(len(win)) - starts[key]
            cores.append((sel, slab, win, loc, slot_within))
            counts_per_core.append(kcounts)

    counts_max = np.max(np.stack(counts_per_core), axis=0).reshape(NWIN, SLABS)
    counts_max = counts_max.max(axis=1)  # worst slab per window
    chunks_per_window = np.maximum(1, -(-counts_max // SLOTS)).astype(np.int64)
    nwt = int(chunks_per_window.sum())
    entry0 = np.concatenate([[0], np.cumsum(chunks_per_window)[:-1]])
    nwt_p = -(-nwt // 128) * 128

    iota = np.tile(np.arange(W, dtype=np.float16), (128, 1))

    in_maps = []
    for (sel, slab, win, loc, slot_within) in cores:
        chunk = slot_within // SLOTS
        slot = (SLOTS * slab + slot_within % SLOTS).astype(np.int64)
        entry = entry0[win] + chunk
        wt = np.zeros((nwt, 128, C), dtype=np.float16)
        idxc = np.full((nwt_p, 128), -1.0, dtype=np.float32)
        if len(sel):
            wt[entry, slot] = vf[sel].astype(np.float16)
            idxc[entry, slot] = loc.astype(np.float32)
        w_dev = np.ascontiguousarray(
            wt.transpose(1, 0, 2).reshape(128, nwt * C))
        idx_dev = np.ascontiguousarray(idxc.T)
        in_maps.append({"w": w_dev, "idx": idx_dev, "iota": iota})

    return in_maps, tuple(int(c) for c in chunks_per_window), nwt


def _run(voxel_features, coords, trace=False):
    from concourse.bass_utils import run_bass_kernel_spmd

    in_maps, chunks, nwt = _host_pack(voxel_features, coords)
    key = chunks
    if key not in _cache:
        _cache[key] = _build_program(chunks, nwt)
    nc = _cache[key]

    res = run_bass_kernel_spmd(nc, in_maps, core_ids=list(range(N_CORES)),
                               trace=trace)
    out = np.zeros((B, C, NY, NX), dtype=np.float32)
    for k in range(N_CORES):
        b, g = divmod(k, 2)
        canvas = _unscramble(res.results[k]["out"].reshape(-1))
        out[b, :, g * HALF_Y : (g + 1) * HALF_Y, :] = canvas.reshape(
            C, HALF_Y, NX)
    return out, res


def kernel(voxel_features, coords, batch_size=B):
    assert int(batch_size) == B
    out, _ = _run(voxel_features, coords, trace=False)
    return out


# revision 4
# speedup vs baseline: 1.2718x; 1.2164x over previous
"""PointPillarsScatter on 8 TRN2 NeuronCores.

Reference op: scatter N pillar feature vectors [N, 64] into a canvas
[B=4, C=64, NY=496, NX=432] at (y, x) cell coords (zero elsewhere).

Sharding: 8 cores = 4 batches x 2 y-halves. Core k=(b, g) owns the
canvas slice out[b, :, 248*g : 248*(g+1), :] -> flat [64, 107136].

Device algorithm (per core), all standard engine ops:
  - canvas produced in column-windows of W=512 cells across 2 column-
    slabs stacked on partitions: window tile [128, 512], partition
    p = 64*a + c (a = slab, c = channel).
  - host packs the <=48 pillars per (window, slab) into slots living on
    SBUF partitions [0,48) (slab 0) and [64,112) (slab 1) — bases chosen
    to satisfy the PE base-partition alignment (0/32/64).  Dense fp16
    weights w[slot, e*64 + c] = feat[pillar, c]; f32 idx[slot, e] = the
    pillar's column in [0, 512) (or -1 for empty slots).
  - DVE builds onehot[k, j] = (iota[j] == idx[k, e]) in one fp16
    tensor_scalar per window (ints < 2048 are exact in fp16; the f32
    scalar compare is exact; fp16 all-SBUF operands hit the 4x DVE mode).
  - PE: one matmul per slab, lhsT = dense fp16 weights [48, 64], rhs =
    onehot [48, 512], into PSUM partitions [64a, 64a+64).  fp16 streams
    1 cycle/column vs 4 for fp32, and the dense lhsT removes any
    block-diagonal weight expansion.  Products are exact fp16 values
    widened in f32 PSUM; each canvas cell gets exactly one product, so
    occupied cells equal fp16(feature) (rel err ~2^-11 << 2e-2 gate)
    and empty cells exact 0.0.
  - blocks of up to 4 windows accumulate in one [128, 2048] PSUM tile
    (4 banks); a single ACT copy moves the block to SBUF, and the DMA is
    issued from ACT right behind it (no cross-engine hop) to a
    CONTIGUOUS DRAM block at full 8KB-descriptor line rate.  The block
    schedule ramps [1,1,2,4,...] so the output stream starts while the
    weight DMAs still occupy the engines, and two dummy warm-up matmuls
    bring the PE out of its low-clock p-state before the real stream.
  - host unscrambles the blocks into the final canvas layout.

Self-contained: shapes hardcoded, no sibling imports.
"""

import numpy as np

NY, NX, C = 496, 432, 64
B = 4
N_CORES = 8
HALF_Y = NY // 2  # 248
CORE_COLS = HALF_Y * NX  # 107136 canvas cells per core
SLABS = 2
SLAB = CORE_COLS // SLABS  # 53568
W = 512  # window width (canvas cells per matmul)
NWIN = (SLAB + W - 1) // W  # 105 windows (last = 320 cols)
LAST_W = SLAB - (NWIN - 1) * W  # 320
SL = 48  # pillar slots per slab per chunk
PBASE = (0, 64)  # slot partition base per slab (matmul alignment)
NPART = 112  # partitions spanned by oh/idx tiles
OUT_ELEMS = C * CORE_COLS  # per-core output element count
SIZES = [1, 1, 2] + [4] * 25  # windows per output block (+ last 320 win)

_cache = {}


def _build_program(chunks_per_window, nwt, *, oh_bufs=6, sb_bufs=3, g0=16,
                   psum_bufs=2, warmup=2, mode="full"):
    """Build the shared SPMD bass program for the given window schedule.

    chunks_per_window: list[int] of length NWIN (>=1 each), shared by all
    cores. nwt == sum(chunks_per_window) weight-tile entries.
    """
    import concourse.bacc as bacc
    import concourse.bass as bass
    import concourse.tile as tile
    import concourse.mybir as mybir
    from contextlib import ExitStack

    f32 = mybir.dt.float32
    f16 = mybir.dt.float16

    nc = bacc.Bacc("TRN2", target_bir_lowering=False, debug=False,
                   num_devices=N_CORES)
    nwt_p = -(-nwt // 128) * 128
    g0 = min(g0, nwt)
    # w rows: slab*48 + slot (96 rows), entry-major fp16 features
    w_dram = nc.dram_tensor("w", [SLABS * SL, nwt * C], f16,
                            kind="ExternalInput")
    idx_dram = nc.dram_tensor("idx", [NPART, nwt_p], f32,
                              kind="ExternalInput")
    out_dram = nc.dram_tensor("out", [1, OUT_ELEMS], f32,
                              kind="ExternalOutput")

    sizes = SIZES
    assert sum(sizes) == NWIN - 1

    with tile.TileContext(nc) as tc, ExitStack() as ctx:
        const_pool = ctx.enter_context(tc.tile_pool(name="const", bufs=1))
        w_pool = ctx.enter_context(tc.tile_pool(name="wpool", bufs=1))
        oh_pool = ctx.enter_context(tc.tile_pool(name="ohpool", bufs=oh_bufs))
        out_pool = ctx.enter_context(tc.tile_pool(name="opool", bufs=sb_bufs))
        psum_pool = ctx.enter_context(
            tc.tile_pool(name="pspool", bufs=psum_bufs, space="PSUM"))

        idx_t = const_pool.tile([NPART, nwt_p], f32, name="idx_t")
        nc.sync.dma_start(idx_t[:], idx_dram.ap())
        iota_t = const_pool.tile([NPART, W], f16, name="iota_t")
        nc.gpsimd.iota(iota_t[:], [[1, W]], channel_multiplier=0,
                       allow_small_or_imprecise_dtypes=True)

        # weight tiles: wt0 = entries [0, g0) (small, unblocks the ramp),
        # wt1 = entries [g0, nwt) in one big line-rate DMA per slab
        wt0 = w_pool.tile([NPART, g0 * C], f16, name="wt0")
        for a in range(SLABS):
            src = bass.AP(w_dram, a * SL * nwt * C,
                          [[nwt * C, SL], [1, g0 * C]])
            nc.sync.dma_start(wt0[PBASE[a] : PBASE[a] + SL, :], src)
        rest = nwt - g0
        wt1 = None
        if rest:
            wt1 = w_pool.tile([NPART, rest * C], f16, name="wt1")
            for a in range(SLABS):
                src = bass.AP(w_dram, a * SL * nwt * C + g0 * C,
                              [[nwt * C, SL], [1, rest * C]])
                nc.sync.dma_start(wt1[PBASE[a] : PBASE[a] + SL, :], src)

        if warmup:
            # dummy matmuls on the iota tile: pull PE out of the low-clock
            # p-state before the first real window arrives
            wps = psum_pool.tile([128, 2048], f32, tag="ps", name="warm_ps")
            for _ in range(warmup):
                nc.tensor.matmul(wps[0:64, :W], iota_t[0:SL, 0:C],
                                 iota_t[0:SL, :W], start=True, stop=True)

        e = 0

        def do_window(ps, j0, w, n):
            nonlocal e
            nchunks = chunks_per_window[w]
            for t in range(nchunks):
                if e < g0:
                    wt, woff = wt0, e * C
                else:
                    wt, woff = wt1, (e - g0) * C
                oh = oh_pool.tile([NPART, W], f16, tag="oh", name=f"oh_{w}_{t}")
                nc.vector.tensor_scalar(
                    oh[:, :n], iota_t[:, :n], idx_t[:, e : e + 1], None,
                    op0=mybir.AluOpType.is_equal)
                for a in range(SLABS):
                    pb = PBASE[a]
                    nc.tensor.matmul(
                        ps[C * a : C * a + C, j0 : j0 + n],
                        wt[pb : pb + SL, woff : woff + C],
                        oh[pb : pb + SL, :n],
                        start=(t == 0), stop=(t == nchunks - 1))
                e += 1

        w = 0
        off = 0
        for bi, q in enumerate(sizes):
            qn = q * W
            ps = psum_pool.tile([128, 2048], f32, tag="ps", name=f"ps_{bi}")
            for wl in range(q):
                do_window(ps, wl * W, w, W)
                w += 1
            sb = out_pool.tile([128, 2048], f32, tag="sb", name=f"sb_{bi}")
            nc.scalar.copy(sb[:, :qn], ps[:, :qn])
            if mode == "full":
                dst = bass.AP(out_dram, off, [[qn, 128], [1, qn]])
                nc.scalar.dma_start(dst, sb[:, :qn])
            off += 128 * qn
        # remainder 320-col window
        ps = psum_pool.tile([128, 2048], f32, tag="ps", name="ps_rem")
        do_window(ps, 0, w, LAST_W)
        w += 1
        sb = out_pool.tile([128, 2048], f32, tag="sb", name="sb_rem")
        nc.scalar.copy(sb[:, :LAST_W], ps[:, :LAST_W])
        if mode == "full":
            dst = bass.AP(out_dram, off, [[LAST_W, 128], [1, LAST_W]])
            nc.scalar.dma_start(dst, sb[:, :LAST_W])
        off += 128 * LAST_W
        assert w == NWIN and e == nwt and off == OUT_ELEMS
    nc.compile()
    return nc


def _unscramble(core_flat):
    """[OUT_ELEMS] scrambled ramp blocks -> canvas [C, CORE_COLS]."""
    canvas = np.empty((C, CORE_COLS), dtype=np.float32)
    canvas_v = canvas.reshape(C, SLABS, SLAB)
    off = 0
    w0 = 0
    for q in SIZES:
        qn = q * W
        blk = core_flat[off : off + 128 * qn].reshape(SLABS, C, qn)
        canvas_v[:, :, w0 * W : w0 * W + qn] = blk.transpose(1, 0, 2)
        off += 128 * qn
        w0 += q
    blk = core_flat[off : off + 128 * LAST_W].reshape(SLABS, C, LAST_W)
    canvas_v[:, :, w0 * W : w0 * W + LAST_W] = blk.transpose(1, 0, 2)
    return canvas


def _host_pack(voxel_features, coords):
    """Shard + pack inputs for the 8 cores.

    Returns (in_maps, chunks_per_window, nwt).
    """
    vf = np.asarray(voxel_features, dtype=np.float32)
    cd = np.asarray(coords)
    bidx = cd[:, 0].astype(np.int64)
    yy = cd[:, 2].astype(np.int64)
    xx = cd[:, 3].astype(np.int64)

    # jax scatter drops out-of-bounds indices; match by masking them out
    inb = (yy >= 0) & (yy < NY) & (xx >= 0) & (xx < NX)

    cores = []
    counts_per_core = []
    for b in range(B):
        for g in range(2):
            sel = np.nonzero(inb & (bidx == b) & (yy >= g * HALF_Y)
                             & (yy < (g + 1) * HALF_Y))[0]
            flat = (yy[sel] - g * HALF_Y) * NX + xx[sel]  # [0, CORE_COLS)
            # dedupe duplicate cells, keep the LAST occurrence
            if len(flat):
                u_rev, first_rev = np.unique(flat[::-1], return_index=True)
                keep = len(flat) - 1 - first_rev
                sel, flat = sel[keep], flat[keep]
            slab = flat // SLAB
            within = flat % SLAB
            win = within // W
            loc = within % W
            key = win * SLABS + slab
            order = np.argsort(key, kind="stable")
            sel, slab, win, loc = sel[order], slab[order], win[order], loc[order]
            key = key[order]
            kcounts = np.bincount(key, minlength=NWIN * SLABS)
            starts = np.concatenate([[0], np.cumsum(kcounts)[:-1]])
            slot_within = np.arange(len(win)) - starts[key]
            cores.append((sel, slab, win, loc, slot_within))
            counts_per_core.append(kcounts)

    counts_max = np.max(np.stack(counts_per_core), axis=0).reshape(NWIN, SLABS)
    counts_max = counts_max.max(axis=1)  # worst slab per window
    chunks_per_window = np.maximum(1, -(-counts_max // SL)).astype(np.int64)
    nwt = int(chunks_per_window.sum())
    entry0 = np.concatenate([[0], np.cumsum(chunks_per_window)[:-1]])
    nwt_p = -(-nwt // 128) * 128

    in_maps = []
    for (sel, slab, win, loc, slot_within) in cores:
        chunk = slot_within // SL
        sw = slot_within % SL
        entry = entry0[win] + chunk
        wrow = (SL * slab + sw).astype(np.int64)       # [0, 96)
        irow = (np.asarray(PBASE)[slab] + sw).astype(np.int64)  # 0-47/64-111
        wt = np.zeros((nwt, SLABS * SL, C), dtype=np.float16)
        idxc = np.full((nwt_p, NPART), -1.0, dtype=np.float32)
        if len(sel):
            wt[entry, wrow] = vf[sel].astype(np.float16)
            idxc[entry, irow] = loc.astype(np.float32)
        w_dev = np.ascontiguousarray(
            wt.transpose(1, 0, 2).reshape(SLABS * SL, nwt * C))
        idx_dev = np.ascontiguousarray(idxc.T)
        in_maps.append({"w": w_dev, "idx": idx_dev})

    return in_maps, tuple(int(c) for c in chunks_per_window), nwt


def _run(voxel_features, coords, trace=False):
    from concourse.bass_utils import run_bass_kernel_spmd

    in_maps, chunks, nwt = _host_pack(voxel_features, coords)
    key = chunks
    if key not in _cache:
        _cache[key] = _build_program(chunks, nwt)
    nc = _cache[key]

    res = run_bass_kernel_spmd(nc, in_maps, core_ids=list(range(N_CORES)),
                               trace=trace)
    out = np.zeros((B, C, NY, NX), dtype=np.float32)
    for k in range(N_CORES):
        b, g = divmod(k, 2)
        canvas = _unscramble(res.results[k]["out"].reshape(-1))
        out[b, :, g * HALF_Y : (g + 1) * HALF_Y, :] = canvas.reshape(
            C, HALF_Y, NX)
    return out, res


def kernel(voxel_features, coords, batch_size=B):
    assert int(batch_size) == B
    out, _ = _run(voxel_features, coords, trace=False)
    return out


# revision 7
# speedup vs baseline: 1.2907x; 1.0148x over previous
"""PointPillarsScatter on 8 TRN2 NeuronCores.

Reference op: scatter N pillar feature vectors [N, 64] into a canvas
[B=4, C=64, NY=496, NX=432] at (y, x) cell coords (zero elsewhere).

Sharding: 8 cores = 4 batches x 2 y-halves. Core k=(b, g) owns the
canvas slice out[b, :, 248*g : 248*(g+1), :] -> flat [64, 107136].

Device algorithm (per core), all standard engine ops:
  - canvas produced in column-windows of W=512 cells across 2 column-
    slabs stacked on partitions: window tile [128, 512], partition
    p = 64*a + c (a = slab, c = channel).
  - host packs the <=48 pillars per (window, slab) into slots living on
    SBUF partitions [0,48) (slab 0) and [64,112) (slab 1) — bases chosen
    to satisfy the PE base-partition alignment (0/32/64).  Dense fp16
    weights w[slot, e*64 + c] = feat[pillar, c]; f32 idx[slot, e] = the
    pillar's column in [0, 512) (or -1 for empty slots).
  - DVE builds onehot[k, j] = (iota[j] == idx[k, e]) in one fp16
    tensor_scalar per window (ints < 2048 are exact in fp16; the f32
    scalar compare is exact; fp16 all-SBUF operands hit the 4x DVE mode).
  - PE: one matmul per slab, lhsT = dense fp16 weights [48, 64], rhs =
    onehot [48, 512], into PSUM partitions [64a, 64a+64).  fp16 streams
    1 cycle/column vs 4 for fp32, and the dense lhsT removes any
    block-diagonal weight expansion.  Products are exact fp16 values
    widened in f32 PSUM; each canvas cell gets exactly one product, so
    occupied cells equal fp16(feature) (rel err ~2^-11 << 2e-2 gate)
    and empty cells exact 0.0.
  - blocks of up to 4 windows accumulate in one [128, 2048] PSUM tile
    (4 banks); a single ACT copy moves the block to SBUF, and the DMA is
    issued from ACT right behind it (no cross-engine hop) to a
    CONTIGUOUS DRAM block at full 8KB-descriptor line rate.  The block
    schedule ramps [1,1,2,4,...] so the output stream starts while the
    weight DMAs still occupy the engines, and two dummy warm-up matmuls
    bring the PE out of its low-clock p-state before the real stream.
  - host unscrambles the blocks into the final canvas layout.

Self-contained: shapes hardcoded, no sibling imports.
"""

import numpy as np

NY, NX, C = 496, 432, 64
B = 4
N_CORES = 8
HALF_Y = NY // 2  # 248
CORE_COLS = HALF_Y * NX  # 107136 canvas cells per core
SLABS = 2
SLAB = CORE_COLS // SLABS  # 53568
W = 512  # window width (canvas cells per matmul)
NWIN = (SLAB + W - 1) // W  # 105 windows (last = 320 cols)
LAST_W = SLAB - (NWIN - 1) * W  # 320
SL = 48  # pillar slots per slab per chunk
PBASE = (0, 64)  # slot partition base per slab (matmul alignment)
NPART = 112  # partitions spanned by oh/idx tiles
OUT_ELEMS = C * CORE_COLS  # per-core output element count
SIZES = [2, 3, 3] + [4] * 24  # windows per output block (+ last 320 win)

_cache = {}


def _build_program(chunks_per_window, nwt, *, oh_bufs=6, sb_bufs=3, g0=16,
                   psum_bufs=2, warmup=2, mode="full"):
    """Build the shared SPMD bass program for the given window schedule.

    chunks_per_window: list[int] of length NWIN (>=1 each), shared by all
    cores. nwt == sum(chunks_per_window) weight-tile entries.
    """
    import concourse.bacc as bacc
    import concourse.bass as bass
    import concourse.tile as tile
    import concourse.mybir as mybir
    from contextlib import ExitStack

    f32 = mybir.dt.float32
    f16 = mybir.dt.float16

    nc = bacc.Bacc("TRN2", target_bir_lowering=False, debug=False,
                   num_devices=N_CORES)
    nwt_p = -(-nwt // 128) * 128
    g0 = min(g0, nwt)
    # w rows: slab*48 + slot (96 rows), entry-major fp16 features
    w_dram = nc.dram_tensor("w", [SLABS * SL, nwt * C], f16,
                            kind="ExternalInput")
    idx_dram = nc.dram_tensor("idx", [NPART, nwt_p], f32,
                              kind="ExternalInput")
    out_dram = nc.dram_tensor("out", [1, OUT_ELEMS], f32,
                              kind="ExternalOutput")

    sizes = SIZES
    assert sum(sizes) == NWIN - 1

    with tile.TileContext(nc) as tc, ExitStack() as ctx:
        const_pool = ctx.enter_context(tc.tile_pool(name="const", bufs=1))
        w_pool = ctx.enter_context(tc.tile_pool(name="wpool", bufs=1))
        oh_pool = ctx.enter_context(tc.tile_pool(name="ohpool", bufs=oh_bufs))
        out_pool = ctx.enter_context(tc.tile_pool(name="opool", bufs=sb_bufs))
        psum_pool = ctx.enter_context(
            tc.tile_pool(name="pspool", bufs=psum_bufs, space="PSUM"))

        idx_t = const_pool.tile([NPART, nwt_p], f32, name="idx_t")
        nc.sync.dma_start(idx_t[:], idx_dram.ap())
        iota_t = const_pool.tile([NPART, W], f16, name="iota_t")
        nc.gpsimd.iota(iota_t[:], [[1, W]], channel_multiplier=0,
                       allow_small_or_imprecise_dtypes=True)

        # weight tiles: wt0 = entries [0, g0) (small, unblocks the ramp),
        # wt1 = entries [g0, nwt) in one big line-rate DMA per slab
        wt0 = w_pool.tile([NPART, g0 * C], f16, name="wt0")
        for a in range(SLABS):
            src = bass.AP(w_dram, a * SL * nwt * C,
                          [[nwt * C, SL], [1, g0 * C]])
            nc.sync.dma_start(wt0[PBASE[a] : PBASE[a] + SL, :], src)
        rest = nwt - g0
        wt1 = None
        if rest:
            wt1 = w_pool.tile([NPART, rest * C], f16, name="wt1")
            for a in range(SLABS):
                src = bass.AP(w_dram, a * SL * nwt * C + g0 * C,
                              [[nwt * C, SL], [1, rest * C]])
                nc.sync.dma_start(wt1[PBASE[a] : PBASE[a] + SL, :], src)

        if warmup:
            # dummy matmuls on the iota tile: pull PE out of the low-clock
            # p-state before the first real window arrives
            wps = psum_pool.tile([128, 2048], f32, tag="ps", name="warm_ps")
            for _ in range(warmup):
                nc.tensor.matmul(wps[0:64, :W], iota_t[0:SL, 0:C],
                                 iota_t[0:SL, :W], start=True, stop=True)

        e = 0

        def do_window(ps, j0, w, n):
            nonlocal e
            nchunks = chunks_per_window[w]
            for t in range(nchunks):
                if e < g0:
                    wt, woff = wt0, e * C
                else:
                    wt, woff = wt1, (e - g0) * C
                oh = oh_pool.tile([NPART, W], f16, tag="oh", name=f"oh_{w}_{t}")
                nc.vector.tensor_scalar(
                    oh[:, :n], iota_t[:, :n], idx_t[:, e : e + 1], None,
                    op0=mybir.AluOpType.is_equal)
                for a in range(SLABS):
                    pb = PBASE[a]
                    nc.tensor.matmul(
                        ps[C * a : C * a + C, j0 : j0 + n],
                        wt[pb : pb + SL, woff : woff + C],
                        oh[pb : pb + SL, :n],
                        start=(t == 0), stop=(t == nchunks - 1))
                e += 1

        w = 0
        off = 0
        for bi, q in enumerate(sizes):
            qn = q * W
            ps = psum_pool.tile([128, 2048], f32, tag="ps", name=f"ps_{bi}")
            for wl in range(q):
                do_window(ps, wl * W, w, W)
                w += 1
            sb = out_pool.tile([128, 2048], f32, tag="sb", name=f"sb_{bi}")
            nc.scalar.copy(sb[:, :qn], ps[:, :qn])
            if mode == "full":
                dst = bass.AP(out_dram, off, [[qn, 128], [1, qn]])
                nc.scalar.dma_start(dst, sb[:, :qn])
            off += 128 * qn
        # remainder 320-col window
        ps = psum_pool.tile([128, 2048], f32, tag="ps", name="ps_rem")
        do_window(ps, 0, w, LAST_W)
        w += 1
        sb = out_pool.tile([128, 2048], f32, tag="sb", name="sb_rem")
        nc.scalar.copy(sb[:, :LAST_W], ps[:, :LAST_W])
        if mode == "full":
            dst = bass.AP(out_dram, off, [[LAST_W, 128], [1, LAST_W]])
            nc.scalar.dma_start(dst, sb[:, :LAST_W])
        off += 128 * LAST_W
        assert w == NWIN and e == nwt and off == OUT_ELEMS
    nc.compile()
    return nc


def _unscramble(core_flat):
    """[OUT_ELEMS] scrambled ramp blocks -> canvas [C, CORE_COLS]."""
    canvas = np.empty((C, CORE_COLS), dtype=np.float32)
    canvas_v = canvas.reshape(C, SLABS, SLAB)
    off = 0
    w0 = 0
    for q in SIZES:
        qn = q * W
        blk = core_flat[off : off + 128 * qn].reshape(SLABS, C, qn)
        canvas_v[:, :, w0 * W : w0 * W + qn] = blk.transpose(1, 0, 2)
        off += 128 * qn
        w0 += q
    blk = core_flat[off : off + 128 * LAST_W].reshape(SLABS, C, LAST_W)
    canvas_v[:, :, w0 * W : w0 * W + LAST_W] = blk.transpose(1, 0, 2)
    return canvas


def _host_pack(voxel_features, coords):
    """Shard + pack inputs for the 8 cores.

    Returns (in_maps, chunks_per_window, nwt).
    """
    vf = np.asarray(voxel_features, dtype=np.float32)
    cd = np.asarray(coords)
    bidx = cd[:, 0].astype(np.int64)
    yy = cd[:, 2].astype(np.int64)
    xx = cd[:, 3].astype(np.int64)

    # jax scatter drops out-of-bounds indices; match by masking them out
    inb = (yy >= 0) & (yy < NY) & (xx >= 0) & (xx < NX)

    cores = []
    counts_per_core = []
    for b in range(B):
        for g in range(2):
            sel = np.nonzero(inb & (bidx == b) & (yy >= g * HALF_Y)
                             & (yy < (g + 1) * HALF_Y))[0]
            flat = (yy[sel] - g * HALF_Y) * NX + xx[sel]  # [0, CORE_COLS)
            # dedupe duplicate cells, keep the LAST occurrence
            if len(flat):
                u_rev, first_rev = np.unique(flat[::-1], return_index=True)
                keep = len(flat) - 1 - first_rev
                sel, flat = sel[keep], flat[keep]
            slab = flat // SLAB
            within = flat % SLAB
            win = within // W
            loc = within % W
            key = win * SLABS + slab
            order = np.argsort(key, kind="stable")
            sel, slab, win, loc = sel[order], slab[order], win[order], loc[order]
            key = key[order]
            kcounts = np.bincount(key, minlength=NWIN * SLABS)
            starts = np.concatenate([[0], np.cumsum(kcounts)[:-1]])
            slot_within = np.arange(len(win)) - starts[key]
            cores.append((sel, slab, win, loc, slot_within))
            counts_per_core.append(kcounts)

    counts_max = np.max(np.stack(counts_per_core), axis=0).reshape(NWIN, SLABS)
    counts_max = counts_max.max(axis=1)  # worst slab per window
    chunks_per_window = np.maximum(1, -(-counts_max // SL)).astype(np.int64)
    nwt = int(chunks_per_window.sum())
    entry0 = np.concatenate([[0], np.cumsum(chunks_per_window)[:-1]])
    nwt_p = -(-nwt // 128) * 128

    in_maps = []
    for (sel, slab, win, loc, slot_within) in cores:
        chunk = slot_within // SL
        sw = slot_within % SL
        entry = entry0[win] + chunk
        wrow = (SL * slab + sw).astype(np.int64)       # [0, 96)
        irow = (np.asarray(PBASE)[slab] + sw).astype(np.int64)  # 0-47/64-111
        wt = np.zeros((nwt, SLABS * SL, C), dtype=np.float16)
        idxc = np.full((nwt_p, NPART), -1.0, dtype=np.float32)
        if len(sel):
            wt[entry, wrow] = vf[sel].astype(np.float16)
            idxc[entry, irow] = loc.astype(np.float32)
        w_dev = np.ascontiguousarray(
            wt.transpose(1, 0, 2).reshape(SLABS * SL, nwt * C))
        idx_dev = np.ascontiguousarray(idxc.T)
        in_maps.append({"w": w_dev, "idx": idx_dev})

    return in_maps, tuple(int(c) for c in chunks_per_window), nwt


def _run(voxel_features, coords, trace=False):
    from concourse.bass_utils import run_bass_kernel_spmd

    in_maps, chunks, nwt = _host_pack(voxel_features, coords)
    key = chunks
    if key not in _cache:
        _cache[key] = _build_program(chunks, nwt)
    nc = _cache[key]

    res = run_bass_kernel_spmd(nc, in_maps, core_ids=list(range(N_CORES)),
                               trace=trace)
    out = np.zeros((B, C, NY, NX), dtype=np.float32)
    for k in range(N_CORES):
        b, g = divmod(k, 2)
        canvas = _unscramble(res.results[k]["out"].reshape(-1))
        out[b, :, g * HALF_Y : (g + 1) * HALF_Y, :] = canvas.reshape(
            C, HALF_Y, NX)
    return out, res


def kernel(voxel_features, coords, batch_size=B):
    assert int(batch_size) == B
    out, _ = _run(voxel_features, coords, trace=False)
    return out
